# revision 40
# baseline (speedup 1.0000x reference)
"""Causal self-attention (B=8, T=1024, C=1024, H=16) on 8 TRN2 NeuronCores.

Sharding: pure data-parallel over batch — core b computes batch element b
with fully replicated weights (B == n_cores, so no collectives needed).

Per-core dataflow (bf16 matmuls, fp32 accumulation):
  1. x DMAs issued ahead of everything; x cast to bf16 on ScalarE, then
     transposed via PE at 1 cycle/row (half the fp32 cost).
  2. qkvT[i, t]: Q^T/K^T computed *transposed* so per-head tiles [64, T]
     are directly the matmul operands for scores; bqkv folded into the
     PSUM evacuation. Pair g+1's weight DMAs issue at pair g's kt=0 and
     its stage copies run mid-pair, so the matmuls (emitted around the
     odd-head AV burst to fill the PSUM-WAR evac windows) never wait.
  3. V in natural [t, v] layout with a ones-column per head so the
     attention*V matmul also produces the softmax denominators.
  4. Scores computed transposed S^T[k, q] per (head, k-tile); causal
     structure skips k>q tiles; exp on ScalarE (scale=1/8 fused);
     diagonal block masked multiplicatively on DVE.
  5. Y^T[d, q] + sums accumulated in PSUM over k-tiles; evacuated as one
     [65, T] bf16 tile on DVE (ScalarE is the pacing engine in the
     attention phase -- exp work/pair exceeds PE work/pair -- so it must
     carry nothing else). Pairs 0-6 normalize via a DRAM-bounce reciprocal-
     broadcast chain on the gpsimd ring (hidden behind later pairs).
     The last pair normalizes fully on-chip (a DMA hop costs ~2.5us):
     PE transposes turn the sums row into columns, the reciprocal runs
     across 128 DVE lanes, PE transposes back and a K=1 ones-matmul
     broadcasts it across partitions; the odd head's unnormalized values
     are shifted to partitions 64..127 early and normalized in place.
  6. out = Y @ Wproj + bproj with tight contiguous PSUM accumulation
     groups (long-open interleaved groups corrupt on hardware); output
     DMAs round-robin over the sync/scalar/gpsimd rings.

GpSimd is used only for memsets/affine-select and DMA issue: its bulk
tensor ops are ~5x slower than DVE and degrade co-running DVE ops.
"""

import numpy as np

import concourse.tile as tile
from concourse import bacc, mybir
from concourse.bass_utils import run_bass_kernel_spmd
from concourse.masks import make_identity

f32 = mybir.dt.float32
bf16 = mybir.dt.bfloat16
AF = mybir.ActivationFunctionType
ALU = mybir.AluOpType

B, T, C, H, HD = 8, 1024, 1024, 16, 64
P = 128
NT = T // P  # 8 token tiles
NS = C // P  # 8 contraction subtiles
W = 66  # per-head stride in V_sb: [64 vals][1 ones][1 pad]


def _build():
    nc = bacc.Bacc(trn_type="TRN2")
    x_d = nc.dram_tensor("x", (T, C), f32, kind="ExternalInput")
    wqkv_d = nc.dram_tensor("wqkv", (C, 3 * C), f32, kind="ExternalInput")
    bqkv_d = nc.dram_tensor("bqkv", (3 * C,), f32, kind="ExternalInput")
    wproj_d = nc.dram_tensor("wproj", (C, C), f32, kind="ExternalInput")
    bproj_d = nc.dram_tensor("bproj", (C,), f32, kind="ExternalInput")
    out_d = nc.dram_tensor("out", (T, C), f32, kind="ExternalOutput")

    with tile.TileContext(nc) as tc:
        with (
            tc.tile_pool(name="big", bufs=1) as big,
            tc.tile_pool(name="stage", bufs=4) as stage,
            tc.tile_pool(name="xbp", bufs=2) as xbp,
            tc.tile_pool(name="ptp", bufs=4) as ptp,
            tc.tile_pool(name="small", bufs=3) as small,
            tc.tile_pool(name="small1", bufs=1) as small1,
            tc.tile_pool(name="small2", bufs=2) as small2,
            tc.tile_pool(name="outp", bufs=2) as outp,
            tc.tile_pool(name="dramp", bufs=6, space="DRAM") as dramp,
            tc.tile_pool(name="pmm", bufs=2, space="PSUM") as pmm,
            tc.tile_pool(name="psp", bufs=2, space="PSUM") as psp,
            tc.tile_pool(name="pyp", bufs=1, space="PSUM") as pyp,
        ):
            # ---------------- x DMAs first (nothing ahead of them) ---------
            x_r = x_d[:, :].rearrange("(i p) c -> p i c", p=P)
            xsts = []
            for i in range(NT):
                xst = stage.tile([P, C], f32, tag="stage", name=f"xst{i}")
                (nc.sync, nc.scalar, nc.gpsimd)[i % 3].dma_start(xst, x_r[:, i, :])
                xsts.append(xst)

            # ---------------- constants ----------------
            ident = big.tile([P, P], bf16, tag="ident")
            make_identity(nc, ident)
            # causal multiplicative mask for the transposed diagonal block:
            # cmask[k, q] = 1 if q >= k else 0
            cmask = big.tile([P, P], bf16, tag="cmask")
            nc.gpsimd.memset(cmask, 1.0)
            nc.gpsimd.affine_select(
                out=cmask,
                in_=cmask,
                compare_op=ALU.is_ge,
                fill=0.0,
                base=0,
                pattern=[[1, P]],
                channel_multiplier=-1,
            )
            ones_row = big.tile([P, HD], bf16, tag="ones")
            nc.gpsimd.memset(ones_row, 1.0)
            # per-partition bias columns for the Q/K part of qkvT
            bqk_col = big.tile([P, 2 * C // P], f32, tag="bqk")
            nc.gpsimd.dma_start(bqk_col, bqkv_d[: 2 * C].rearrange("(o p) -> p o", p=P))

            wqkv_sb = big.tile([P, NS, 3 * C], bf16, tag="wqkv")
            wq_r = wqkv_d[:, :].rearrange("(s p) i -> p s i", p=P)

            # weight-load staging: DMA issue and SBUF cast-copy are emitted
            # at separate points so the DVE never head-of-line blocks on an
            # in-flight DMA
            wst_pend = {}

            def qk_dma(m):
                st = stage.tile([P, C], f32, tag="stage", name=f"wst{m}")
                st3 = st.rearrange("p (s c) -> p s c", c=P)
                nc.sync.dma_start(st3, wq_r[:, :, m * P : (m + 1) * P])
                wst_pend[m] = st3

            def qk_copy(m):
                nc.vector.tensor_copy(
                    wqkv_sb[:, :, m * P : (m + 1) * P], wst_pend.pop(m)
                )

            # pre-issue the first two Q/K weight loads (sync ring, behind x)
            qk_dma(0)
            qk_dma(C // P)

            # V weight load DMAs split across the gpsimd/sync rings; the
            # cast copies run on DVE after the xT evacuations
            vw_pend = []
            for s in range(NS):
                st = stage.tile([P, C], f32, tag="stage", name=f"vw{s}")
                (nc.gpsimd.dma_start if s < 4 else nc.sync.dma_start)(
                    st, wq_r[:, s, 2 * C : 3 * C]
                )
                vw_pend.append(st)

            # broadcast bias rows (per free-dim column) for V and proj;
            # the 128-descriptor issue goes last on the gpsimd ring
            bias_v = big.tile([P, C], f32, tag="bias_v")
            nc.gpsimd.dma_start(bias_v, bqkv_d[2 * C :][None, :].to_broadcast((P, C)))

            # ---------------- cast + PE transpose -> xT bf16 ---------------
            xt_sb = big.tile([P, NS, T], bf16, tag="xt")
            for i in range(NT):
                xb = xbp.tile([P, C], bf16, tag="xb", name=f"xb{i}")
                if i % 2 == 0:
                    nc.vector.tensor_copy(xb, xsts[i])
                else:
                    nc.scalar.copy(xb, xsts[i])
                tp = pmm.tile([P, T], bf16, tag="pmm", name=f"tp{i}")
                for j in range(NS):
                    nc.tensor.transpose(
                        tp[:, j * P : (j + 1) * P],
                        xb[:, j * P : (j + 1) * P],
                        ident,
                    )
                nc.vector.tensor_copy(
                    xt_sb[:, :, i * P : (i + 1) * P],
                    tp.rearrange("p (j t) -> p j t", t=P),
                )

            # ---------------- Q/K^T tiles (emitted one pair ahead) ---------
            qkt_sb = big.tile([P, 2 * C // P, T], bf16, tag="qkt")
            wproj_sb = big.tile([P, NS, C], bf16, tag="wproj")
            wp_r = wproj_d[:, :].rearrange("(s p) j -> p s j", p=P)

            def emit_qk_mm(m):
                for ch in range(2):
                    ps = pmm.tile([P, 512], f32, tag="pmm", name=f"qk{m}_{ch}")
                    for s in range(NS):
                        nc.tensor.matmul(
                            ps,
                            wqkv_sb[:, s, m * P : (m + 1) * P],
                            xt_sb[:, s, ch * 512 : (ch + 1) * 512],
                            start=(s == 0),
                            stop=(s == NS - 1),
                        )
                    nc.vector.tensor_scalar_add(
                        qkt_sb[:, m, ch * 512 : (ch + 1) * 512],
                        ps,
                        bqk_col[:, m : m + 1],
                    )

            qk_copy(0)
            qk_copy(C // P)
            for s in range(NS):
                nc.vector.tensor_copy(wqkv_sb[:, s, 2 * C : 3 * C], vw_pend[s])
            emit_qk_mm(0)
            emit_qk_mm(C // P)

            # ---------------- V (natural layout, ones-augmented) -----------
            v_sb = [big.tile([P, H * W], bf16, tag=f"v{i}", name=f"v{i}") for i in range(NT)]

            def emit_v(i):
                v3 = v_sb[i].rearrange("p (h w) -> p h w", w=W)
                nc.gpsimd.memset(v3[:, :, HD : HD + 1], 1.0)
                for ch in range(2):
                    ps = pmm.tile([P, 512], f32, tag="pmm", name=f"v{i}_{ch}")
                    for s in range(NS):
                        nc.tensor.matmul(
                            ps,
                            xt_sb[:, s, i * P : (i + 1) * P],
                            wqkv_sb[:, s, 2 * C + ch * 512 : 2 * C + (ch + 1) * 512],
                            start=(s == 0),
                            stop=(s == NS - 1),
                        )
                    nc.vector.tensor_tensor(
                        v3[:, 8 * ch : 8 * ch + 8, 0:HD],
                        ps.rearrange("p (h d) -> p h d", d=HD),
                        bias_v[:, ch * 512 : (ch + 1) * 512].rearrange(
                            "p (h d) -> p h d", d=HD
                        ),
                        ALU.add,
                    )

            # ---------------- attention ----------------
            yt_sb = [big.tile([P, T], bf16, tag=f"yt{g}", name=f"yt{g}") for g in range(NT)]

            def s_matmuls(sp, kt_h, qt_h, kt):
                q0 = kt * P
                if kt <= 3:
                    nc.tensor.matmul(
                        sp[:, q0:512], kt_h[:, q0 : q0 + P], qt_h[:, q0:512],
                        start=True, stop=True,
                    )
                    nc.tensor.matmul(
                        sp[:, 512:T], kt_h[:, q0 : q0 + P], qt_h[:, 512:T],
                        start=True, stop=True,
                    )
                else:
                    nc.tensor.matmul(
                        sp[:, q0:T], kt_h[:, q0 : q0 + P], qt_h[:, q0:T],
                        start=True, stop=True,
                    )

            def av_matmuls(ypA, ypB, pt_ap, h, kt, q_off):
                q0 = kt * P
                lhsT_v = v_sb[kt][:, h * W : h * W + HD + 1]  # [128, 65]
                if kt <= 3:
                    nc.tensor.matmul(
                        ypA[0 : HD + 1, q0:512], lhsT_v, pt_ap[:, q0 - q_off : 512 - q_off],
                        start=(kt == 0), stop=(kt == 3),
                    )
                    nc.tensor.matmul(
                        ypB[0 : HD + 1, 0:512], lhsT_v, pt_ap[:, 512 - q_off : T - q_off],
                        start=(kt == 0), stop=(kt == NT - 1),
                    )
                else:
                    nc.tensor.matmul(
                        ypB[0 : HD + 1, q0 - 512 : 512], lhsT_v, pt_ap[:, q0 - q_off : T - q_off],
                        start=False, stop=(kt == NT - 1),
                    )

            def evac_head(ypA, ypB, h):
                # one [65, T] bf16 tile: rows 0..63 = unnormalized Y,
                # row 64 = softmax denominators; DVE takes the low half,
                # ScalarE (idle at pair boundaries) the high half, so the
                # Y psum tile frees fast
                yu = small.tile([HD + 1, T], bf16, tag="yu", name=f"yu{h}")
                nc.vector.tensor_copy(yu[:, 0:512], ypA[0 : HD + 1, 0:512])
                nc.vector.tensor_copy(yu[:, 512:T], ypB[0 : HD + 1, 0:512])
                return (yu, h)

            def norm_chain(state):
                # DRAM-bounce reciprocal-broadcast chain on the gpsimd ring;
                # used for pairs 0-6, where it hides behind later pairs
                yu, h = state
                ring = nc.gpsimd
                scr = dramp.tile([T], bf16, tag="scr", name=f"scr{h}")
                ring.dma_start(scr[None, :], yu[HD : HD + 1, :])
                s64 = small2.tile([HD, T // HD], bf16, tag="s64", name=f"s64_{h}")
                ring.dma_start(s64, scr.rearrange("(p e) -> p e", p=HD))
                r64 = small2.tile([HD, T // HD], bf16, tag="r64", name=f"r64_{h}")
                with nc.allow_low_precision("softmax recips in bf16 (tol 2e-2)"):
                    nc.vector.reciprocal(r64, s64)
                scr2 = dramp.tile([T], bf16, tag="scr2", name=f"scr2_{h}")
                ring.dma_start(scr2.rearrange("(p e) -> p e", p=HD), r64)
                r_sb = small.tile([P, T], bf16, tag="r", name=f"r{h}", bufs=2)
                ring.dma_start(r_sb[0:HD, :], scr2[None, :].to_broadcast((HD, T)))
                return r_sb

            def norm_head(state):
                yu, h = state
                g = h // 2
                r_sb = norm_chain(state)
                if h % 2 == 0:
                    nc.vector.tensor_tensor(
                        yt_sb[g][0:HD, :], yu[0:HD, :], r_sb[0:HD, :], ALU.mult
                    )
                else:
                    ytmp = small1.tile([HD, T], bf16, tag="ytmp", name=f"ytmp{h}")
                    nc.vector.tensor_tensor(ytmp, yu[0:HD, :], r_sb[0:HD, :], ALU.mult)
                    # partition shift 0..63 -> 64..127 via SBUF-to-SBUF DMA
                    nc.gpsimd.dma_start(yt_sb[g][HD:P, :], ytmp)

            bias_o = bias_v  # reused once the V phase is done
            out_r = out_d[:, :].rearrange("(i p) j -> p i j", p=P)

            def proj_group(i, ch):
                ps = pmm.tile([P, 512], f32, tag="pmm", name=f"proj{i}_{ch}")
                for g2 in range(NT):
                    nc.tensor.matmul(
                        ps,
                        yt_sb[g2][:, i * P : (i + 1) * P],
                        wproj_sb[:, g2, ch * 512 : (ch + 1) * 512],
                        start=(g2 == 0),
                        stop=(g2 == NT - 1),
                    )
                ot = outp.tile([P, 512], f32, tag="out")
                nc.vector.tensor_tensor(
                    ot, ps, bias_o[:, ch * 512 : (ch + 1) * 512], ALU.add
                )
                ring = (nc.sync, nc.scalar)[(2 * i + ch) % 2]
                ring.dma_start(out_r[:, i, ch * 512 : (ch + 1) * 512], ot)

            def norm_onchip_t1(state):
                # sums row [1, T] (partition 64) -> columns [128, NT] via PE
                # transposes, then a 128-lane reciprocal on DVE
                yu, h = state
                tc_ps = pmm.tile([P, 2 * NT], bf16, tag="pmm", name=f"tc{h}")
                for j in range(NT):
                    # even columns only: PSUM writes must be 4-byte aligned
                    nc.tensor.transpose(
                        tc_ps[:, 2 * j : 2 * j + 1],
                        yu[HD : HD + 1, j * P : (j + 1) * P],
                        ident[HD : HD + 1, HD : HD + 1],
                    )
                scol = small2.tile([P, NT], bf16, tag="scol", name=f"scol{h}")
                nc.vector.tensor_copy(
                    scol, tc_ps.rearrange("p (j two) -> p j two", two=2)[:, :, 0]
                )
                rcol = small2.tile([P, NT], bf16, tag="rcol", name=f"rcol{h}")
                with nc.allow_low_precision("softmax recips in bf16 (tol 2e-2)"):
                    nc.vector.reciprocal(rcol, scol)
                return rcol

            def norm_onchip_t2(state, rcol):
                # columns back to a row at partition 64, K=1 ones-matmul
                # broadcast across 64 partitions, multiply
                yu, h = state
                g = h // 2
                odd = h % 2 == 1
                lo, hi = (HD, P) if odd else (0, HD)
                rr_ps = psp.tile([P, T], bf16, tag="ps", name=f"rr{h}")
                for j in range(NT):
                    nc.tensor.transpose(
                        rr_ps[HD : HD + 1, j * P : (j + 1) * P],
                        rcol[:, j : j + 1],
                        ident,
                    )
                r_sb = small.tile([P, T], bf16, tag="r", name=f"r{h}", bufs=2)
                nc.scalar.copy(r_sb[HD : HD + 1, :], rr_ps[HD : HD + 1, :])
                rb_ps = pyp.tile([P, T], f32, tag="py", name=f"rb{h}")
                for cch in range(2):
                    nc.tensor.matmul(
                        rb_ps[lo:hi, cch * 512 : (cch + 1) * 512],
                        ones_row[HD : HD + 1, :],
                        r_sb[HD : HD + 1, cch * 512 : (cch + 1) * 512],
                        start=True, stop=True,
                    )
                nc.scalar.copy(r_sb[lo:hi, :], rb_ps[lo:hi, :])
                if not odd:
                    nc.vector.tensor_tensor(
                        yt_sb[g][0:HD, :], yu[0:HD, :], r_sb[0:HD, :], ALU.mult
                    )
                else:
                    nc.vector.tensor_tensor(
                        yt_sb[g][HD:P, :], yt_sb[g][HD:P, :], r_sb[HD:P, :], ALU.mult
                    )

            wpst_pend = []
            pending = []
            for g in range(NT):
                h0, h1 = 2 * g, 2 * g + 1
                m = g
                last = g == NT - 1
                if g == 1:
                    # gpsimd ring: absorbs the 128-descriptor issue without
                    # delaying weight loads on sync or exps on scalar
                    nc.gpsimd.dma_start(
                        bias_o, bproj_d[:][None, :].to_broadcast((P, C))
                    )
                if g == 2:
                    # wproj DMAs issue now; the cast copies are scheduled
                    # into the kt loops of pairs 2-3
                    for s in range(NS):
                        st = stage.tile([P, C], f32, tag="stage", name=f"wpst{s}")
                        (nc.gpsimd.dma_start if s % 2 == 0 else nc.sync.dma_start)(
                            st, wp_r[:, s, :]
                        )
                        wpst_pend.append((s, st))
                qt0 = qkt_sb[0:HD, m, :]
                kt0 = qkt_sb[0:HD, (C // P) + m, :]
                qt1 = qkt_sb[HD:P, m, :]
                kt1 = qkt_sb[HD:P, (C // P) + m, :]
                yp = pyp.tile([P, T], f32, tag="py", name=f"yp{h0}")
                ypA, ypB = yp[:, 0:512], yp[:, 512:T]
                pt1s = []
                for kt in range(NT):
                    if g == 0:
                        emit_v(kt)
                    if not last:
                        if kt == 0:
                            qk_dma(m + 1)
                            qk_dma((C // P) + m + 1)
                        if kt == 2:
                            qk_copy(m + 1)
                        if kt == 5:
                            qk_copy((C // P) + m + 1)
                    if g in (2, 3) and kt >= 2 and wpst_pend and kt < 6:
                        s, st = wpst_pend.pop(0)
                        nc.vector.tensor_copy(wproj_sb[:, s, :], st)
                    if kt in (1, 4) and pending:
                        norm_head(pending.pop(0))
                    q0 = kt * P
                    sp0 = psp.tile([P, T], f32, tag="ps", name=f"sp0_{g}_{kt}")
                    sp1 = psp.tile([P, T], f32, tag="ps", name=f"sp1_{g}_{kt}")
                    s_matmuls(sp0, kt0, qt0, kt)
                    s_matmuls(sp1, kt1, qt1, kt)
                    pt0 = ptp.tile([P, T], bf16, tag="pt", name=f"pt0_{g}_{kt}")
                    nc.scalar.activation(pt0[:, q0:T], sp0[:, q0:T], AF.Exp, scale=0.125)
                    pt1 = small1.tile([P, T - q0], bf16, tag=f"pt1_{kt}", name=f"pt1_{g}_{kt}")
                    nc.scalar.activation(pt1, sp1[:, q0:T], AF.Exp, scale=0.125)
                    # mask the diagonal block (k > q within the block -> 0)
                    nc.vector.tensor_tensor(
                        pt0[:, q0 : q0 + P], pt0[:, q0 : q0 + P], cmask, ALU.mult
                    )
                    nc.vector.tensor_tensor(
                        pt1[:, 0:P], pt1[:, 0:P], cmask, ALU.mult
                    )
                    av_matmuls(ypA, ypB, pt0, h0, kt, 0)
                    pt1s.append(pt1)
                # next pair's Q matmuls fill the even-evac WAR window
                if not last:
                    emit_qk_mm(m + 1)
                st_e = evac_head(ypA, ypB, h0)
                yp1 = pyp.tile([P, T], f32, tag="py", name=f"yp{h1}")
                yp1A, yp1B = yp1[:, 0:512], yp1[:, 512:T]
                for kt in range(NT):
                    av_matmuls(yp1A, yp1B, pt1s[kt], h1, kt, kt * P)
                st_o = evac_head(yp1A, yp1B, h1)
                if last:
                    # ship unnormalized odd-head values to partitions 64..127
                    # immediately; the in-place normalize lands there
                    nc.sync.dma_start(yt_sb[g][HD:P, :], st_o[0][0:HD, :])
                    rc_e = norm_onchip_t1(st_e)
                    rc_o = norm_onchip_t1(st_o)
                    norm_onchip_t2(st_e, rc_e)
                    norm_onchip_t2(st_o, rc_o)
                else:
                    emit_qk_mm((C // P) + m + 1)
                    pending.append(st_e)
                    pending.append(st_o)

            while pending:
                norm_head(pending.pop(0))

            # ---------------- output projection ----------------
            for i in range(NT):
                for ch in range(2):
                    proj_group(i, ch)

    nc.compile()
    return nc


_NC = None


def _get_nc():
    global _NC
    if _NC is None:
        _NC = _build()
    return _NC


def _in_maps(x, Wqkv, bqkv, Wproj, bproj):
    x = np.ascontiguousarray(np.asarray(x, dtype=np.float32))
    shared = {
        "wqkv": np.ascontiguousarray(np.asarray(Wqkv, dtype=np.float32)),
        "bqkv": np.ascontiguousarray(np.asarray(bqkv, dtype=np.float32)),
        "wproj": np.ascontiguousarray(np.asarray(Wproj, dtype=np.float32)),
        "bproj": np.ascontiguousarray(np.asarray(bproj, dtype=np.float32)),
    }
    return [{"x": np.ascontiguousarray(x[b]), **shared} for b in range(B)]


def run(x, Wqkv, bqkv, Wproj, bproj, **run_kwargs):
    """Run on 8 cores; returns (output [B,T,C] fp32, BassKernelResults)."""
    nc = _get_nc()
    res = run_bass_kernel_spmd(
        nc, _in_maps(x, Wqkv, bqkv, Wproj, bproj), core_ids=list(range(B)), **run_kwargs
    )
    out = np.stack([res.results[b]["out"] for b in range(B)]).astype(np.float32)
    return out, res


def kernel(x, Wqkv, bqkv, Wproj, bproj, n_head=None, **_ignored):
    out, _ = run(x, Wqkv, bqkv, Wproj, bproj)
    return out
